# revision 14
# baseline (speedup 1.0000x reference)
"""Trainium2 Bass kernel for a 2-layer LSTM autoregressive rollout (batch=1).

Strategy (8 NeuronCores, SPMD):
- Tensor-parallel split of the 4H=8192 gate dimension: each core owns 1024
  gate rows (= 256 h elements per layer); all big weights live SBUF-resident
  in bf16 ([1024,2048] shards of W_hh0 / W_ih1 / W_hh1 per core).
- Matvecs run on the PE with the h-vector chunk as the stationary operand
  ([128,1] lhsT) and the weight shard as the moving operand ([128,512] bf16),
  accumulating gates into PSUM [1,512] x 2 banks per layer.
- Per-step h exchange uses remote_dma_broadcast (SBUF->SBUF SDMA with
  remote-semaphore arrival signaling), one call per XOR-distance d=1..7,
  writing column pair d of every peer's gather buffer.  Weight shards are
  pre-permuted on the host so that each core's gather-buffer column pair c
  corresponds to the h chunk owned by core (my_id ^ c); the program is fully
  SPMD-symmetric (no core-id registers anywhere).
- Gate nonlinearities: ACT sigmoid/tanh on the narrow PSUM rows, one HWDGE
  scatter DMA ([1,1024] -> [128,8]) into a wide layout, then DVE updates the
  c/h state at 128-lane width.  The fc head is computed as per-core partials
  ([128,14] W-stationary matmuls) that ride along the h1 exchange and are
  reduced on every core, so each core redundantly updates q/qd/delay in f32.
"""

import os
import sys

import numpy as np

for _p in ("/opt/trn_rl_repo",):
    if _p not in sys.path and os.path.isdir(_p):
        sys.path.insert(0, _p)

import ml_dtypes

import concourse.bass as bass
import concourse.bacc as bacc
import concourse.mybir as mybir

BF16 = ml_dtypes.bfloat16

NJ = 7
IN = 2 * NJ + 1  # 15
H = 2048
G4 = 4 * H  # 8192
SCALE = 0.1
DT_NORM = 0.01 / 1.0
STEPS = 256
NCORES = 8
LOC = G4 // NCORES  # 1024 local gate rows per core
HLOC = H // NCORES  # 256 local h elements per core

# local gate type order: 0=i, 1=f, 2=o, 3=g  (sigmoid on types 0..2);
# global torch gate blocks are H-sized: i [0,H), f [H,2H), g [2H,3H), o [3H,4H)
TYPE_BASE = np.array([0, H, 3 * H, 2 * H])

dt = mybir.dt
AF = mybir.ActivationFunctionType
ALU = mybir.AluOpType


def _row_perm(k: int) -> np.ndarray:
    """Global W row index for local gate position lin in [0, 1024).

    lin maps to: scattered-gate buffer (p=lin>>3, c=lin&7), psum half
    r=lin>>9, col s=lin&511.  c>>1 = gate type (i,f,o,g), c&1 = u,
    local h element = 128*u + p.
    """
    lin = np.arange(LOC)
    p = lin >> 3
    c = lin & 7
    t = c >> 1
    u = c & 1
    return TYPE_BASE[t] + HLOC * k + 128 * u + p


def _h_cols(k: int, phi) -> np.ndarray:
    """Global h column index for gather-buffer (p, m) with m=2d+u.

    phi[k][d] = logical id of the core whose chunk lands in column pair d of
    core k's gather buffer (measured by the runtime probe; phi[k][0] == k).
    """
    m = np.arange(2 * NCORES)
    d = m >> 1
    u = m & 1
    p = np.arange(128)
    own = np.array([phi[k][dd] for dd in d])
    return (HLOC * own + 128 * u)[None, :] + p[:, None]  # [128, 16]


def _big_rhs(W: np.ndarray, k: int, phi) -> np.ndarray:
    """[8192, 2048] weight -> per-core moving-operand layout [128, 16384] bf16.

    MM block (m, r) at columns [(m*2+r)*512 : +512], rhs[p, s] =
    W[rowperm[512r+s], hcol(m, p)].
    """
    rp = _row_perm(k)
    hc = _h_cols(k, phi)
    A = W[rp]  # [1024, 2048]
    B = A[:, hc]  # [1024, 128, 16]
    B = B.reshape(2, 512, 128, 16).transpose(2, 3, 0, 1)  # [p, m, r, s]
    return np.ascontiguousarray(B.reshape(128, 16 * 2 * 512).astype(BF16))


def prep_core_inputs(inputs: dict, k: int, phi=None) -> dict:
    if phi is None:
        phi = XOR_PHI
    W_ih0 = np.asarray(inputs["W_ih0"], np.float32)
    W_hh0 = np.asarray(inputs["W_hh0"], np.float32)
    W_ih1 = np.asarray(inputs["W_ih1"], np.float32)
    W_hh1 = np.asarray(inputs["W_hh1"], np.float32)
    b0 = np.asarray(inputs["b_ih0"], np.float32) + np.asarray(inputs["b_hh0"], np.float32)
    b1 = np.asarray(inputs["b_ih1"], np.float32) + np.asarray(inputs["b_hh1"], np.float32)
    fc_W = np.asarray(inputs["fc_W"], np.float32)
    fc_b = np.asarray(inputs["fc_b"], np.float32)
    h_init = np.asarray(inputs["hidden_h"], np.float32)
    c_init = np.asarray(inputs["hidden_c"], np.float32)
    q0 = np.asarray(inputs["current_q"], np.float32)[0, 0]
    qd0 = np.asarray(inputs["current_qd"], np.float32)[0, 0]
    delay0 = float(np.asarray(inputs["start_delay"], np.float32).reshape(()))

    rp = _row_perm(k)

    # W_ih0 augmented with the combined bias as x-row 15 (x_aug[15] == 1.0)
    Waug = np.concatenate([W_ih0, b0[:, None]], axis=1)  # [8192, 16]
    wx = np.ascontiguousarray(Waug[rp].T.astype(np.float32))  # [16, 1024]

    fcw = np.zeros((128, 28), np.float32)
    for u in range(2):
        fcw[:, u * 14:(u + 1) * 14] = \
            fc_W[:, HLOC * k + 128 * u:HLOC * k + 128 * (u + 1)].T

    b1row = b1[rp].reshape(1, LOC).astype(np.float32)

    hb0i = h_init[0, 0][_h_cols(k, phi)].astype(np.float32)  # [128, 16]
    hb1i = np.zeros((128, 24), np.float32)
    for m in range(16):
        d, u = m >> 1, m & 1
        hb1i[:, 3 * d + u] = \
            h_init[1, 0][HLOC * phi[k][d] + 128 * u + np.arange(128)]

    c0i = c_init[0, 0][HLOC * k:HLOC * (k + 1)].reshape(2, 128).T.astype(np.float32)
    c1i = c_init[1, 0][HLOC * k:HLOC * (k + 1)].reshape(2, 128).T.astype(np.float32)

    xi = np.zeros((16, 1), np.float32)
    xi[0:7, 0] = q0
    xi[7:14, 0] = qd0
    xi[14, 0] = delay0
    xi[15, 0] = 1.0

    return {
        "w0": _big_rhs(W_hh0, k, phi),
        "w1i": _big_rhs(W_ih1, k, phi),
        "w1h": _big_rhs(W_hh1, k, phi),
        "wx": wx,
        "fcw": fcw.astype(BF16),
        "b1row": b1row,
        "hb0i": hb0i.astype(BF16),
        "hb1i": hb1i.astype(BF16),
        "c0i": np.ascontiguousarray(c0i),
        "c1i": np.ascontiguousarray(c1i),
        "xi": xi,
        "fcbi": (fc_b * SCALE).reshape(14, 1).astype(np.float32),
        "resci": np.array([0.0] * 14 + [DT_NORM, 0.0],
                          np.float32).reshape(16, 1),
        "onesi": np.ones((1, 1), np.float32),
    }


XOR_PHI = [[r ^ d for d in range(NCORES)] for r in range(NCORES)]


def _build_probe() -> bass.Bass:
    """Tiny SPMD program: each core broadcasts its signature to XOR-distance
    d, revealing which logical core's data lands in each gather column."""
    nc = bacc.Bacc("TRN2", target_bir_lowering=False, debug=False)
    sig_in = nc.dram_tensor("sig", [128, 1], dt.float32, kind="ExternalInput")
    out = nc.dram_tensor("out", [128, 8], dt.float32, kind="ExternalOutput")
    buf = nc.alloc_sbuf_tensor("buf", [128, 8], dt.float32)
    s_in = nc.alloc_semaphore("s_in")
    rsem = nc.alloc_semaphore("rsem")
    lsem = nc.alloc_semaphore("lsem")
    prep = nc.alloc_semaphore("prep")
    outs = nc.alloc_semaphore("outs")

    with nc.Block() as block:

        @block.sync
        def _(sync):
            sync.dma_start(buf[:, 0:1], sig_in[:, :]).then_inc(s_in, 16)
            sync.wait_ge(rsem, 14)
            sync.dma_start(out[:, :], buf[:, :]).then_inc(outs, 16)
            sync.wait_ge(outs, 16)

        @block.gpsimd
        def _(gp):
            gp.wait_ge(s_in, 16)
            for d in range(1, 8):
                rd = [None] * 8
                rd[d] = (0, d)
                gp.remote_dma_broadcast(
                    buf[:, d:d + 1], buf[:, 0:1],
                    remote_sem=rsem, local_sem=lsem,
                    rdests=rd, queue_num=0).then_inc(prep, 1)
            gp.wait_ge(prep, 7)
            gp.trigger_dma(count=7, queue_num=0)

    nc.compile()
    return nc


_PHI = None


def measure_phi():
    """Measure phi[r][d] on the attached hardware (cached per process)."""
    global _PHI
    if _PHI is not None:
        return _PHI
    try:
        from concourse.bass_utils import run_bass_kernel_spmd

        nc = _build_probe()
        in_maps = [{"sig": np.full((128, 1), float(k), np.float32)}
                   for k in range(NCORES)]
        res = run_bass_kernel_spmd(nc, in_maps, core_ids=list(range(NCORES)))
        phi = []
        for r in range(NCORES):
            row = np.round(res.results[r]["out"][0]).astype(int).tolist()
            assert sorted(row) == list(range(NCORES)) and row[0] == r, row
            phi.append(row)
        _PHI = phi
    except Exception:
        _PHI = XOR_PHI
    return _PHI


def build(steps: int = STEPS, trn_type: str = "TRN2") -> bass.Bass:
    nc = bacc.Bacc(trn_type, target_bir_lowering=False, debug=False)

    # ---- DRAM I/O ----
    din = {
        "w0": nc.dram_tensor("w0", [128, 16384], dt.bfloat16, kind="ExternalInput"),
        "w1i": nc.dram_tensor("w1i", [128, 16384], dt.bfloat16, kind="ExternalInput"),
        "w1h": nc.dram_tensor("w1h", [128, 16384], dt.bfloat16, kind="ExternalInput"),
        "wx": nc.dram_tensor("wx", [16, 1024], dt.float32, kind="ExternalInput"),
        "fcw": nc.dram_tensor("fcw", [128, 28], dt.bfloat16, kind="ExternalInput"),
        "b1row": nc.dram_tensor("b1row", [1, 1024], dt.float32, kind="ExternalInput"),
        "hb0i": nc.dram_tensor("hb0i", [128, 16], dt.bfloat16, kind="ExternalInput"),
        "hb1i": nc.dram_tensor("hb1i", [128, 24], dt.bfloat16, kind="ExternalInput"),
        "c0i": nc.dram_tensor("c0i", [128, 2], dt.float32, kind="ExternalInput"),
        "c1i": nc.dram_tensor("c1i", [128, 2], dt.float32, kind="ExternalInput"),
        "xi": nc.dram_tensor("xi", [16, 1], dt.float32, kind="ExternalInput"),
        "fcbi": nc.dram_tensor("fcbi", [14, 1], dt.float32, kind="ExternalInput"),
        "resci": nc.dram_tensor("resci", [16, 1], dt.float32, kind="ExternalInput"),
        "onesi": nc.dram_tensor("onesi", [1, 1], dt.float32, kind="ExternalInput"),
    }
    out_x = nc.dram_tensor("out_x", [16, 1], dt.float32, kind="ExternalOutput")
    out_h = nc.dram_tensor("out_h", [128, 4], dt.float32, kind="ExternalOutput")
    out_c = nc.dram_tensor("out_c", [128, 4], dt.float32, kind="ExternalOutput")

    # ---- SBUF ----
    w0 = nc.alloc_sbuf_tensor("w0_sb", [128, 16384], dt.bfloat16)
    w1i = nc.alloc_sbuf_tensor("w1i_sb", [128, 16384], dt.bfloat16)
    w1h = nc.alloc_sbuf_tensor("w1h_sb", [128, 16384], dt.bfloat16)
    wx = nc.alloc_sbuf_tensor("wx_sb", [16, 1024], dt.float32)
    fcw = nc.alloc_sbuf_tensor("fcw_sb", [128, 28], dt.bfloat16)
    b1row = nc.alloc_sbuf_tensor("b1row_sb", [1, 1024], dt.float32)
    ones = nc.alloc_sbuf_tensor("ones_sb", [1, 1], dt.float32)
    hb0 = [nc.alloc_sbuf_tensor(f"hb0_{p}", [128, 16], dt.bfloat16) for p in range(2)]
    hb1 = [nc.alloc_sbuf_tensor(f"hb1_{p}", [128, 24], dt.bfloat16) for p in range(2)]
    sg0 = nc.alloc_sbuf_tensor("sg0_sb", [1, 1024], dt.float32)
    sg1 = nc.alloc_sbuf_tensor("sg1_sb", [1, 1024], dt.float32)
    gsc0 = nc.alloc_sbuf_tensor("gsc0_sb", [128, 8], dt.float32)
    gsc1 = nc.alloc_sbuf_tensor("gsc1_sb", [128, 8], dt.float32)
    c0 = nc.alloc_sbuf_tensor("c0_sb", [128, 2], dt.float32)
    c1 = nc.alloc_sbuf_tensor("c1_sb", [128, 2], dt.float32)
    tc0 = nc.alloc_sbuf_tensor("tc0_sb", [128, 2], dt.float32)
    tc1 = nc.alloc_sbuf_tensor("tc1_sb", [128, 2], dt.float32)
    tB0 = nc.alloc_sbuf_tensor("tB0_sb", [128, 2], dt.float32)
    tB1 = nc.alloc_sbuf_tensor("tB1_sb", [128, 2], dt.float32)
    xv = nc.alloc_sbuf_tensor("x_sb", [16, 1], dt.float32)
    fcsum = nc.alloc_sbuf_tensor("fcsum_sb", [14, 1], dt.float32)
    res = nc.alloc_sbuf_tensor("res_sb", [16, 1], dt.float32)
    fcb = nc.alloc_sbuf_tensor("fcb_sb", [14, 1], dt.float32)
    hf32 = nc.alloc_sbuf_tensor("hf32_sb", [128, 4], dt.float32)

    # ---- PSUM ----
    pg0 = [nc.alloc_psum_tensor(f"pg0{r}", [1, 512], dt.float32) for r in range(2)]
    pg1 = [nc.alloc_psum_tensor(f"pg1{r}", [1, 512], dt.float32) for r in range(2)]
    pfc = nc.alloc_psum_tensor("pfc", [14, 1], dt.float32)

    # ---- semaphores ----
    sem = {
        n: nc.alloc_semaphore(n)
        for n in [
            "s_in", "pe_g0", "pe_g1", "pe_fc",
            "act0", "act1", "atc0", "atc1",
            "dve_c0", "dve_c1", "dve_h0", "dve_h1", "dve_fcc", "dve_x",
            "dve_p0", "dve_p1", "dve_r1", "dve_r2",
            "sc0", "sc1", "prep", "rsem0", "rsem1", "lsem0", "lsem1", "outs",
        ]
    }

    sb_loads = [
        (w0, din["w0"]), (w1i, din["w1i"]), (w1h, din["w1h"]), (wx, din["wx"]),
        (fcw, din["fcw"]), (b1row, din["b1row"]), (hb0[1], din["hb0i"]),
        (hb1[1], din["hb1i"]), (c0, din["c0i"]), (c1, din["c1i"]),
        (xv, din["xi"]), (fcb, din["fcbi"]), (ones, din["onesi"]),
        (res, din["resci"]),
    ]
    IN_CNT = len(sb_loads) * 16

    with nc.Block() as block:

        @block.sync
        def _(sync):
            for sb, dr in sb_loads:
                sync.dma_start(sb[:, :], dr[:, :]).then_inc(sem["s_in"], 16)
            # per-step scatter DMAs
            for t in range(steps):
                sync.wait_ge(sem["act0"], t + 1)
                if t >= 1:
                    sync.wait_ge(sem["dve_h0"], t)  # gsc0 reads of step t-1 done
                sync.dma_start(gsc0[:, :], sg0[:, :]).then_inc(sem["sc0"], 16)
                sync.wait_ge(sem["act1"], t + 1)
                if t >= 1:
                    sync.wait_ge(sem["dve_h1"], t)
                sync.dma_start(gsc1[:, :], sg1[:, :]).then_inc(sem["sc1"], 16)
            # final outputs
            sync.wait_ge(sem["dve_x"], steps)
            sync.dma_start(out_x[:, :], xv[:, :]).then_inc(sem["outs"], 16)
            sync.wait_ge(sem["dve_fcc"], steps)  # after both hf32 writes
            sync.dma_start(out_h[:, :], hf32[:, :]).then_inc(sem["outs"], 16)
            sync.wait_ge(sem["dve_c1"], steps)
            sync.dma_start(out_c[:, 0:2], c0[:, :]).then_inc(sem["outs"], 16)
            sync.dma_start(out_c[:, 2:4], c1[:, :]).then_inc(sem["outs"], 16)
            sync.wait_ge(sem["outs"], 64)

        @block.tensor
        def _(te):
            te.wait_ge(sem["s_in"], IN_CNT)
            for t in range(steps):
                pi = t & 1
                rho = 1 - pi
                # ---- gates0: W_hh0 @ h0_{t-1}, then W_ih0aug @ x_t ----
                if t >= 1:
                    te.wait_ge(sem["act0"], t)        # pg0 free
                    te.wait_ge(sem["rsem0"], 14 * t)  # h0_{t-1} arrivals
                for r in range(2):
                    for m in range(16):
                        te.matmul(
                            pg0[r][0:1, :], hb0[rho][:, m:m + 1],
                            w0[:, (m * 2 + r) * 512:(m * 2 + r + 1) * 512],
                            start=(m == 0), stop=False, skip_group_check=True)
                if t >= 1:
                    te.wait_ge(sem["dve_x"], t)
                for r in range(2):
                    ins = te.matmul(pg0[r][0:1, :], xv[:, 0:1],
                                    wx[:, r * 512:(r + 1) * 512],
                                    start=False, stop=True, skip_group_check=True)
                    if r == 1:
                        ins.then_inc(sem["pe_g0"], 1)
                # ---- gates1: bias, W_hh1 @ h1_{t-1}, then W_ih1 @ h0_t ----
                if t >= 1:
                    te.wait_ge(sem["act1"], t)
                    te.wait_ge(sem["rsem1"], 14 * t)
                for r in range(2):
                    te.matmul(pg1[r][0:1, :], ones[0:1, 0:1],
                              b1row[0:1, r * 512:(r + 1) * 512],
                              start=True, stop=False, skip_group_check=True)
                    for m in range(16):
                        col = 3 * (m >> 1) + (m & 1)
                        te.matmul(
                            pg1[r][0:1, :], hb1[rho][:, col:col + 1],
                            w1h[:, (m * 2 + r) * 512:(m * 2 + r + 1) * 512],
                            start=False, stop=False, skip_group_check=True)
                te.wait_ge(sem["dve_h0"], t + 1)
                for r in range(2):
                    for m in range(2):
                        te.matmul(
                            pg1[r][0:1, :], hb0[pi][:, m:m + 1],
                            w1i[:, (m * 2 + r) * 512:(m * 2 + r + 1) * 512],
                            start=False, stop=False, skip_group_check=True)
                te.wait_ge(sem["rsem0"], 14 * (t + 1))
                for r in range(2):
                    for m in range(2, 16):
                        ins = te.matmul(
                            pg1[r][0:1, :], hb0[pi][:, m:m + 1],
                            w1i[:, (m * 2 + r) * 512:(m * 2 + r + 1) * 512],
                            start=False, stop=(m == 15), skip_group_check=True)
                        if r == 1 and m == 15:
                            ins.then_inc(sem["pe_g1"], 1)
                # ---- fc partial ----
                if t >= 1:
                    te.wait_ge(sem["dve_fcc"], t)  # pfc free
                te.wait_ge(sem["dve_h1"], t + 1)
                te.matmul(pfc[0:14, 0:1], fcw[:, 0:14], hb1[pi][:, 0:1],
                          start=True, stop=False, skip_group_check=True)
                te.matmul(pfc[0:14, 0:1], fcw[:, 14:28], hb1[pi][:, 1:2],
                          start=False, stop=True,
                          skip_group_check=True).then_inc(sem["pe_fc"], 1)

        @block.scalar
        def _(act):
            act.wait_ge(sem["s_in"], IN_CNT)
            for t in range(steps):
                for (lname, pg, sg, semn, dvec, tcn, cn, atcn, scn) in (
                    ("l0", pg0, sg0, "act0", "dve_c0", tc0, c0, "atc0", "sc0"),
                    ("l1", pg1, sg1, "act1", "dve_c1", tc1, c1, "atc1", "sc1"),
                ):
                    act.wait_ge(sem["pe_g0" if lname == "l0" else "pe_g1"], t + 1)
                    if t >= 1:  # sg free: previous scatter DMA has read it
                        act.wait_ge(sem[scn], 16 * t)
                    for r in range(2):
                        pgr = pg[r][0:1, :].rearrange("p (a b) -> p a b", b=8)
                        sgr = sg[0:1, r * 512:(r + 1) * 512].rearrange(
                            "p (a b) -> p a b", b=8)
                        act.activation(sgr[:, :, 0:6], pgr[:, :, 0:6], AF.Sigmoid)
                        ins = act.activation(sgr[:, :, 6:8], pgr[:, :, 6:8], AF.Tanh)
                        if r == 1:
                            ins.then_inc(sem[semn], 1)
                    act.wait_ge(sem[dvec], t + 1)
                    act.activation(tcn[:, :], cn[:, :], AF.Tanh).then_inc(sem[atcn], 1)

        @block.vector
        def _(dve):
            dve.wait_ge(sem["s_in"], IN_CNT)
            # partitions 14..127 of the parity-0 fc-partial column are never
            # written by compute but are read by the h1 broadcast: zero once
            dve.memset(hb1[0][:, 2:3], 0.0)
            for t in range(steps):
                pi = t & 1
                for (lname, gsc, cn, tBn, tcn, hb, scn, pn, cnn, atcn, hn, lsn) in (
                    ("l0", gsc0, c0, tB0, tc0, hb0, "sc0", "dve_p0", "dve_c0",
                     "atc0", "dve_h0", "lsem0"),
                    ("l1", gsc1, c1, tB1, tc1, hb1, "sc1", "dve_p1", "dve_c1",
                     "atc1", "dve_h1", "lsem1"),
                ):
                    dve.wait_ge(sem[scn], 16 * (t + 1))
                    # tB = sigma(i) * tanh(g)
                    dve.tensor_mul(tBn[:, :], gsc[:, 0:2],
                                   gsc[:, 6:8]).then_inc(sem[pn], 1)
                    # own-pipeline drain (dve_p0 gets 2 incs/step: tB + clip)
                    dve.wait_ge(sem[pn], (2 * t + 1) if lname == "l0" else (t + 1))
                    for u in range(2):
                        ins = dve.scalar_tensor_tensor(
                            cn[:, u:u + 1], cn[:, u:u + 1], gsc[:, 2 + u:3 + u],
                            tBn[:, u:u + 1], ALU.mult, ALU.add)
                        if u == 1:
                            ins.then_inc(sem[cnn], 1)
                    dve.wait_ge(sem[atcn], t + 1)
                    if t >= 2:
                        dve.wait_ge(sem[lsn], 112 * (t - 1))  # hb sends done
                    dve.tensor_mul(hb[pi][:, 0:2], gsc[:, 4:6],
                                   tcn[:, :]).then_inc(sem[hn], 1)
                    if t == steps - 1:
                        off = 0 if lname == "l0" else 2
                        dve.tensor_mul(hf32[:, off:off + 2], gsc[:, 4:6], tcn[:, :])
                # fc partial into the h1 payload
                dve.wait_ge(sem["pe_fc"], t + 1)
                dve.tensor_copy(hb1[pi][0:14, 2:3],
                                pfc[0:14, 0:1]).then_inc(sem["dve_fcc"], 1)
                # after h1 exchange: reduce fc partials, update x
                dve.wait_ge(sem["rsem1"], 14 * (t + 1))
                dve.wait_ge(sem["dve_fcc"], t + 1)  # own col-2 write-back
                fcc = hb1[pi][0:14, 0:24].rearrange("p (d u) -> p d u", u=3)
                dve.tensor_reduce(fcsum[:, :], fcc[:, :, 2], mybir.AxisListType.X,
                                  ALU.add).then_inc(sem["dve_r1"], 1)
                dve.wait_ge(sem["dve_r1"], t + 1)
                dve.scalar_tensor_tensor(res[0:14, 0:1], fcsum[:, :], SCALE,
                                         fcb[0:14, 0:1], ALU.mult,
                                         ALU.add).then_inc(sem["dve_r2"], 1)
                dve.wait_ge(sem["dve_r2"], t + 1)
                dve.tensor_scalar(res[0:14, 0:1], res[0:14, 0:1], -0.2, 0.2,
                                  ALU.max, ALU.min).then_inc(sem["dve_p0"], 1)
                dve.wait_ge(sem["dve_p0"], 2 * t + 2)  # clip write-back
                # rows 14,15 of res hold (DT_NORM, 0): one add updates q/qd/delay
                dve.tensor_add(xv[:, :], xv[:, :],
                               res[:, :]).then_inc(sem["dve_x"], 1)

        @block.gpsimd
        def _(gp):
            gp.wait_ge(sem["s_in"], IN_CNT)
            for t in range(steps):
                pi = t & 1
                # h0 exchange
                for d in range(1, 8):
                    rd = [None] * 8
                    rd[d] = (0, d)
                    gp.remote_dma_broadcast(
                        hb0[pi][:, 2 * d:2 * d + 2], hb0[pi][:, 0:2],
                        remote_sem=sem["rsem0"], local_sem=sem["lsem0"],
                        rdests=rd, queue_num=0).then_inc(sem["prep"], 1)
                gp.wait_ge(sem["prep"], 14 * t + 7)
                gp.wait_ge(sem["dve_h0"], t + 1)
                gp.trigger_dma(count=7, queue_num=0)
                # h1 exchange (h chunk + fc partial)
                for d in range(1, 8):
                    rd = [None] * 8
                    rd[d] = (0, d)
                    gp.remote_dma_broadcast(
                        hb1[pi][:, 3 * d:3 * d + 3], hb1[pi][:, 0:3],
                        remote_sem=sem["rsem1"], local_sem=sem["lsem1"],
                        rdests=rd, queue_num=0).then_inc(sem["prep"], 1)
                gp.wait_ge(sem["prep"], 14 * t + 14)
                gp.wait_ge(sem["dve_fcc"], t + 1)
                gp.trigger_dma(count=7, queue_num=0)

    nc.compile()
    return nc


def assemble_outputs(results: list[dict]):
    q = results[0]["out_x"][0:7, 0].astype(np.float32)
    qd = results[0]["out_x"][7:14, 0].astype(np.float32)
    h = np.zeros((2, 1, H), np.float32)
    c = np.zeros((2, 1, H), np.float32)
    for k in range(NCORES):
        oh = results[k]["out_h"]
        oc = results[k]["out_c"]
        for u in range(2):
            sl = slice(HLOC * k + 128 * u, HLOC * k + 128 * (u + 1))
            h[0, 0, sl] = oh[:, u]
            h[1, 0, sl] = oh[:, 2 + u]
            c[0, 0, sl] = oc[:, u]
            c[1, 0, sl] = oc[:, 2 + u]
    return (q.reshape(1, 1, NJ), qd.reshape(1, 1, NJ), h, c)


_CACHED = {}


def _get_program(steps: int):
    if steps not in _CACHED:
        _CACHED[steps] = build(steps)
    return _CACHED[steps]


def kernel(**inputs) -> tuple:
    steps = int(inputs.get("steps", STEPS))
    phi = measure_phi()
    nc = _get_program(steps)
    in_maps = [prep_core_inputs(inputs, k, phi) for k in range(NCORES)]
    from concourse.bass_utils import run_bass_kernel_spmd

    out = run_bass_kernel_spmd(nc, in_maps, core_ids=list(range(NCORES)))
    return assemble_outputs(out.results)


if __name__ == "__main__":
    nc = build(4)
    print("built ok")
